# revision 14
# baseline (speedup 1.0000x reference)
"""Laplace attention kernel for Trainium2 (8 NeuronCores, SPMD data-parallel).

Reference computation (per batch b):
    unnorm[i,j] = sum_d |(k[j,d] - v[i,d]) * 0.5|
    weights     = softmax_j(unnorm)          # rows i, softmax over j
    out[i,:]    = sum_j weights[i,j] * v[j,:]

B=8 batches -> one batch per NeuronCore, no cross-core communication.

Per-core algorithm (M=512, D=64, P=128):
  - Layouts:  vT2 [128=(t,d), 512=i] bf16 : v transposed, duplicated over t
              k2T [128=(t,d), 256=mj] f32 : column mj = [k[2mj,:]; k[2mj+1,:]]
  - For each j-pair mj: one DVE tensor_scalar
        absd[(t,d), i] = abs_max(vT2 - k2T[:,mj], 0) = |v[i,d] - k[2mj+t,d]|
    then one TensorE matmul with a constant selector lhsT [128,2]
    (column t selects the 64 d-rows of half t) reducing over d:
        unnT[2r+t, i] += ... -> PSUM bank q holds unnT rows j=128q..128q+127
    This produces unnorm TRANSPOSED ([j,i]), which is exactly the lhsT the
    final matmul needs -- no weight transpose anywhere.
  - Softmax without max-subtraction (values bounded, fp32 exp is safe):
        wT[j,i] = exp(0.5 * unnT[j,i])  (one ACT op per bank, PSUM->SBUF bf16)
  - Final matmul with v augmented by a ones column:
        out_aug[i, 0:64] = sum_j wT[j,i] * v[j,:],  out_aug[i,64] = sum_j wT[j,i]
    then out = out_aug[:, 0:64] * (1 / out_aug[:, 64]).
"""

import os

import numpy as np

M = 512
D = 64
B = 8
P = 128
NB = M // P  # 4 row-blocks
NMJ = M // 2  # 256 j-pairs
# Global shift on the softmax logits so exp() fits fp16 range (~[2e-14, 6e4]).
# Logits 0.5*sum_d|k-v| lie in ~[21, 55] for these inputs; weights are stored
# as exp(logit - 45); numerator and denominator scale identically so softmax
# ratios are unchanged.
EXP_SHIFT = 38.0

_CACHE = {}


def _build_module():
    import concourse.mybir as mybir
    import concourse.tile as tile
    from concourse import bacc

    nc = bacc.Bacc("TRN2", target_bir_lowering=False, debug=False,
                   enable_asserts=False)
    k_dram = nc.dram_tensor("k", [M, D], mybir.dt.float32, kind="ExternalInput")
    v_dram = nc.dram_tensor("v", [M, D], mybir.dt.float32, kind="ExternalInput")
    out_dram = nc.dram_tensor("out", [M, D], mybir.dt.float32,
                              kind="ExternalOutput")

    with tile.TileContext(nc) as tc:
        _emit(tc, nc, k_dram.ap(), v_dram.ap(), out_dram.ap())
    nc.compile()
    return nc


def _emit(tc, nc, k, v, out):
    from contextlib import ExitStack

    import concourse.mybir as mybir
    from concourse.masks import make_identity

    f32 = mybir.dt.float32
    fp16 = mybir.dt.float16
    bf16 = mybir.dt.bfloat16
    Alu = mybir.AluOpType
    Act = mybir.ActivationFunctionType

    ctx = ExitStack()
    const = ctx.enter_context(tc.tile_pool(name="const", bufs=1))
    absd_pool = ctx.enter_context(tc.tile_pool(name="absd", bufs=4))
    wt_pool = ctx.enter_context(tc.tile_pool(name="wt", bufs=2))
    small = ctx.enter_context(tc.tile_pool(name="small", bufs=1))
    psum_tr = ctx.enter_context(tc.tile_pool(name="psum_tr", bufs=2,
                                             space="PSUM"))
    psum_unn = ctx.enter_context(tc.tile_pool(name="psum_unn", bufs=2,
                                              space="PSUM"))
    psum_out = ctx.enter_context(tc.tile_pool(name="psum_out", bufs=1,
                                              space="PSUM"))

    # ---- prep: identity for PE transposes -------------------------------
    ident = const.tile([P, P], f32, name="ident")
    make_identity(nc, ident)

    # ---- k2T [128, 256] f32: transpose of k.reshape(256, 128) -----------
    k2_view = k.rearrange("(m t) d -> m (t d)", t=2)  # [256, 128] reshape view
    k2T = const.tile([P, NMJ], f32, name="k2T")
    for h in range(2):
        k2h = const.tile([P, P], f32, name=f"k2_{h}")
        nc.sync.dma_start(k2h[:], k2_view[h * P:(h + 1) * P, :])
        ptr = psum_tr.tile([P, P], f32, name=f"ptr_k_{h}", tag="ptr")
        nc.tensor.transpose(ptr[:], k2h[:], ident[:])
        nc.scalar.copy(k2T[:, h * P:(h + 1) * P], ptr[:])

    # ---- K1[j] = sum_d k[j,d], laid out [1, 512] as (t, mj) -------------
    # Needed because |a-b| = 2*max(a,b) - a - b:
    #   logit[j,i] = 0.5*sum_d|k-v| = Smax[j,i] - 0.5*K1[j] - 0.5*V1[i]
    # and the V1[i] term is constant along each softmax column -> cancels.
    half_sel = const.tile([P, 2], f32, name="half_sel")
    nc.gpsimd.memset(half_sel[:], 0.0)
    nc.gpsimd.memset(half_sel[0:D, 0:1], 1.0)
    nc.gpsimd.memset(half_sel[D:2 * D, 1:2], 1.0)
    k1_psum = psum_tr.tile([1, M], f32, name="k1_psum", tag="ptr")
    for t in range(2):
        nc.tensor.matmul(k1_psum[0:1, t * NMJ:(t + 1) * NMJ],
                         half_sel[:, t:t + 1], k2T[:],
                         start=True, stop=True)
    # Copy PSUM (t-major) -> SBUF j-major [1, 512]: out position 2*mj + t.
    k1_rows = const.tile([1, M], f32, name="k1_rows")
    nc.scalar.copy(k1_rows.rearrange("p (mj t) -> p t mj", t=2)[:],
                   k1_psum.rearrange("p (t mj) -> p t mj", t=2)[:])
    # Per-bank j-major [128, 1] bias columns via PE transpose.
    bias_col = []
    for q in range(NB):
        trc = psum_tr.tile([P, 1], f32, name=f"trc_{q}", tag="ptr")
        nc.tensor.transpose(trc[:], k1_rows[0:1, q * P:(q + 1) * P],
                            ident[0:1, 0:1])
        bc = const.tile([P, 1], f32, name=f"bias_{q}")
        nc.vector.tensor_scalar(bc[:], trc[:], -0.5, -EXP_SHIFT,
                                op0=Alu.mult, op1=Alu.add)
        bias_col.append(bc)

    # ---- v tiles, vT2 [128, 512] bf16, v_aug [128, 65] bf16 x4 ----------
    vT2 = const.tile([P, M], fp16, name="vT2")
    v_aug = []
    for q in range(NB):
        vq = const.tile([P, D], f32, name=f"v_{q}")
        nc.sync.dma_start(vq[:], v[q * P:(q + 1) * P, :])
        ptv = psum_tr.tile([D, P], f32, name=f"ptr_v_{q}", tag="ptr")
        nc.tensor.transpose(ptv[:], vq[:], ident[:])
        # copy vT block into both t-halves of vT2 (cast f32 -> bf16)
        nc.scalar.copy(vT2[0:D, q * P:(q + 1) * P], ptv[:])
        nc.scalar.copy(vT2[D:2 * D, q * P:(q + 1) * P], ptv[:])
        va = const.tile([P, D + 1], bf16, name=f"v_aug_{q}")
        nc.vector.tensor_copy(va[:, 0:D], vq[:])
        nc.gpsimd.memset(va[:, D:D + 1], 1.0)
        v_aug.append(va)

    # ---- selector band [128, 132] bf16 -----------------------------------
    # band[c, y] = 1 iff y == 64 + (c >= 64).  The lhsT for local pair m is
    # band[:, 64-2m : 128-2m]: a [128, 64] matrix whose column p is 1 exactly
    # when p == 2m + t(c), t(c) = c // 64 -- so the matmul adds the d-sum of
    # half t of absd into output row 2m+t.  One static tensor, 32 shifted
    # views; matmul PSUM writes stay at legal base partitions {0, 64}.
    band = const.tile([P, 132], fp16, name="band")
    nc.gpsimd.memset(band[:], 0.0)
    nc.gpsimd.memset(band[0:D, D:D + 1], 1.0)
    nc.gpsimd.memset(band[D:2 * D, D + 1:D + 2], 1.0)


    # ---- main: distance + softmax + weighted sum ------------------------
    out_aug = [psum_out.tile([P, D + 1], f32, name=f"out_aug_{qp}")
               for qp in range(NB)]

    for q in range(NB):
        unn = psum_unn.tile([P, M], f32, name=f"unn_{q}", tag="unn")
        for h in range(2):  # half-bank: 64 j-rows at partition offset 64*h
            for m in range(32):
                mj = q * 64 + h * 32 + m
                absd = absd_pool.tile([P, M], fp16, name="absd", tag="absd")
                nc.vector.tensor_scalar(
                    absd[:], vT2[:], k2T[:, mj:mj + 1], None,
                    op0=Alu.max)
                nc.tensor.matmul(
                    unn[D * h:D * h + D, :], band[:, D - 2 * m:2 * D - 2 * m],
                    absd[:], start=(m == 0), stop=(m == 31))
        wT = wt_pool.tile([P, M], bf16, name="wT", tag="wT")
        nc.scalar.activation(wT[:], unn[:], Act.Exp, scale=1.0,
                             bias=bias_col[q][:])
        for qp in range(NB):
            nc.tensor.matmul(
                out_aug[qp][:], wT[:, qp * P:(qp + 1) * P], v_aug[q][:],
                start=(q == 0), stop=(q == NB - 1), skip_group_check=True)

    # ---- normalize + store ----------------------------------------------
    for qp in range(NB):
        recip = small.tile([P, 1], f32, name=f"recip_{qp}")
        nc.vector.reciprocal(recip[:], out_aug[qp][:, D:D + 1])
        res = small.tile([P, D], f32, name=f"res_{qp}")
        nc.vector.tensor_scalar(
            res[:], out_aug[qp][:, 0:D], recip[:], None, op0=Alu.mult)
        nc.sync.dma_start(out[qp * P:(qp + 1) * P, :], res[:])

    ctx.close()


def _get_module():
    if "nc" not in _CACHE:
        _CACHE["nc"] = _build_module()
    return _CACHE["nc"]


def _run(k, v, trace=False):
    """k, v: [B, M, D] f32. Returns (out [B, M, D] f32, BassKernelResults)."""
    from concourse import bass_utils

    nc = _get_module()
    in_maps = [
        {"k": np.ascontiguousarray(k[b], dtype=np.float32),
         "v": np.ascontiguousarray(v[b], dtype=np.float32)}
        for b in range(B)
    ]
    res = bass_utils.run_bass_kernel_spmd(
        nc, in_maps, core_ids=list(range(B)), trace=trace)
    out = np.stack([res.results[b]["out"] for b in range(B)], axis=0)
    return out, res


def kernel(**inputs):
    k = np.asarray(inputs["k"])
    v = np.asarray(inputs["v"])
    out, _ = _run(k, v, trace=bool(int(os.environ.get("KERNEL_TRACE", "0"))))
    return out.astype(np.float32)


# revision 16
# speedup vs baseline: 1.0538x; 1.0538x over previous
"""Laplace attention kernel for Trainium2 (8 NeuronCores, SPMD data-parallel).

Reference computation (per batch b):
    unnorm[i,j] = sum_d |(k[j,d] - v[i,d]) * 0.5|
    weights     = softmax_j(unnorm)          # rows i, softmax over j
    out[i,:]    = sum_j weights[i,j] * v[j,:]

B=8 batches -> one batch per NeuronCore, no cross-core communication.

Per-core algorithm (M=512, D=64, P=128):
  - Layouts:  vT2 [128=(t,d), 512=i] bf16 : v transposed, duplicated over t
              k2T [128=(t,d), 256=mj] f32 : column mj = [k[2mj,:]; k[2mj+1,:]]
  - For each j-pair mj: one DVE tensor_scalar
        absd[(t,d), i] = abs_max(vT2 - k2T[:,mj], 0) = |v[i,d] - k[2mj+t,d]|
    then one TensorE matmul with a constant selector lhsT [128,2]
    (column t selects the 64 d-rows of half t) reducing over d:
        unnT[2r+t, i] += ... -> PSUM bank q holds unnT rows j=128q..128q+127
    This produces unnorm TRANSPOSED ([j,i]), which is exactly the lhsT the
    final matmul needs -- no weight transpose anywhere.
  - Softmax without max-subtraction (values bounded, fp32 exp is safe):
        wT[j,i] = exp(0.5 * unnT[j,i])  (one ACT op per bank, PSUM->SBUF bf16)
  - Final matmul with v augmented by a ones column:
        out_aug[i, 0:64] = sum_j wT[j,i] * v[j,:],  out_aug[i,64] = sum_j wT[j,i]
    then out = out_aug[:, 0:64] * (1 / out_aug[:, 64]).
"""

import os

import numpy as np

M = 512
D = 64
B = 8
P = 128
NB = M // P  # 4 row-blocks
NMJ = M // 2  # 256 j-pairs
# Global shift on the softmax logits so exp() fits fp16 range (~[2e-14, 6e4]).
# Logits 0.5*sum_d|k-v| lie in ~[21, 55] for these inputs; weights are stored
# as exp(logit - 45); numerator and denominator scale identically so softmax
# ratios are unchanged.
EXP_SHIFT = 38.0

_CACHE = {}

# Experiment knobs (overridden by exp harness): dtype of the distance-phase
# 16-bit tensors and of the TS scalar operand.
CFG = {"mx_dt": "float16", "ts_imm": False}


def _build_module(cfg=None):
    import concourse.mybir as mybir
    import concourse.tile as tile
    from concourse import bacc

    nc = bacc.Bacc("TRN2", target_bir_lowering=False, debug=False,
                   enable_asserts=False)
    k_dram = nc.dram_tensor("k", [M, D], mybir.dt.float32, kind="ExternalInput")
    v_dram = nc.dram_tensor("v", [M, D], mybir.dt.float32, kind="ExternalInput")
    out_dram = nc.dram_tensor("out", [M, D], mybir.dt.float32,
                              kind="ExternalOutput")

    with tile.TileContext(nc) as tc:
        _emit(tc, nc, k_dram.ap(), v_dram.ap(), out_dram.ap(), cfg or CFG)
    nc.compile()
    return nc


def _emit(tc, nc, k, v, out, cfg):
    from contextlib import ExitStack

    import concourse.mybir as mybir
    from concourse.masks import make_identity

    f32 = mybir.dt.float32
    fp16 = getattr(mybir.dt, cfg.get("mx_dt", "float16"))
    bf16 = mybir.dt.bfloat16
    Alu = mybir.AluOpType
    Act = mybir.ActivationFunctionType

    ctx = ExitStack()
    const = ctx.enter_context(tc.tile_pool(name="const", bufs=1))
    absd_pool = ctx.enter_context(tc.tile_pool(name="absd", bufs=6))
    act_pool = ctx.enter_context(tc.tile_pool(name="absd_act", bufs=64))
    wt_pool = ctx.enter_context(tc.tile_pool(name="wt", bufs=4))
    small = ctx.enter_context(tc.tile_pool(name="small", bufs=1))
    # PSUM: tr pool (2 banks) is prep-only and released before the main-loop
    # pools (4 + 4 banks) are created, so peak stays at 8 banks.
    tr_ctx = ExitStack()
    psum_tr = tr_ctx.enter_context(tc.tile_pool(name="psum_tr", bufs=2,
                                                space="PSUM"))

    # ---- prep: identity for PE transposes -------------------------------
    ident = const.tile([P, P], f32, name="ident")
    make_identity(nc, ident)

    # ---- k2T [128, 256] f32: transpose of k.reshape(256, 128) -----------
    k2_view = k.rearrange("(h m t) d -> m h (t d)", t=2, h=2)  # [128, 2, 128]
    k2all = const.tile([P, 2, P], f32, name="k2all")
    nc.scalar.dma_start(k2all[:], k2_view[:])
    k2T = const.tile([P, NMJ], f32, name="k2T")
    for h in range(2):
        ptr = psum_tr.tile([P, P], f32, name=f"ptr_k_{h}", tag="ptr")
        nc.tensor.transpose(ptr[:], k2all[:, h, :], ident[:])
        nc.scalar.copy(k2T[:, h * P:(h + 1) * P], ptr[:])
    neg_k2T = const.tile([P, NMJ], f32, name="neg_k2T")
    nc.vector.tensor_scalar(neg_k2T[:], k2T[:], -1.0, None, op0=Alu.mult)

    # ---- K1[j] = sum_d k[j,d], laid out [1, 512] as (t, mj) -------------
    # Needed because |a-b| = 2*max(a,b) - a - b:
    #   logit[j,i] = 0.5*sum_d|k-v| = Smax[j,i] - 0.5*K1[j] - 0.5*V1[i]
    # and the V1[i] term is constant along each softmax column -> cancels.
    half_sel = const.tile([P, 2], f32, name="half_sel")
    nc.gpsimd.memset(half_sel[:], 0.0)
    nc.gpsimd.memset(half_sel[0:D, 0:1], 1.0)
    nc.gpsimd.memset(half_sel[D:2 * D, 1:2], 1.0)
    k1_psum = psum_tr.tile([1, M], f32, name="k1_psum", tag="ptr")
    for t in range(2):
        nc.tensor.matmul(k1_psum[0:1, t * NMJ:(t + 1) * NMJ],
                         half_sel[:, t:t + 1], k2T[:],
                         start=True, stop=True)
    # Copy PSUM (t-major) -> SBUF j-major [1, 512]: out position 2*mj + t.
    k1_rows = const.tile([1, M], f32, name="k1_rows")
    nc.scalar.copy(k1_rows.rearrange("p (mj t) -> p t mj", t=2)[:],
                   k1_psum.rearrange("p (t mj) -> p t mj", t=2)[:])
    # Per-bank j-major [128, 1] bias columns via PE transpose.
    bias_col = []
    for q in range(NB):
        trc = psum_tr.tile([P, 1], f32, name=f"trc_{q}", tag="ptr")
        nc.tensor.transpose(trc[:], k1_rows[0:1, q * P:(q + 1) * P],
                            ident[0:1, 0:1])
        bc = const.tile([P, 1], f32, name=f"bias_{q}")
        sgn = 0.5 if q == NB - 1 else -0.5
        nc.vector.tensor_scalar(bc[:], trc[:], sgn, -EXP_SHIFT,
                                op0=Alu.mult, op1=Alu.add)
        bias_col.append(bc)

    # ---- v tiles, vT2 [128, 512] bf16, v_aug [128, 65] bf16 x4 ----------
    vT2 = const.tile([P, M], fp16, name="vT2")
    v4 = const.tile([P, NB, D], f32, name="v4")
    nc.sync.dma_start(v4[:], v.rearrange("(q p) d -> p q d", p=P)[:])
    v_aug = []
    for q in range(NB):
        vq = v4[:, q, :]
        ptv = psum_tr.tile([D, P], f32, name=f"ptr_v_{q}", tag="ptr")
        nc.tensor.transpose(ptv[:], vq, ident[:])
        # copy vT block into both t-halves of vT2 (cast f32 -> bf16)
        nc.scalar.copy(vT2[0:D, q * P:(q + 1) * P], ptv[:])
        nc.scalar.copy(vT2[D:2 * D, q * P:(q + 1) * P], ptv[:])
        va = const.tile([P, D + 1], bf16, name=f"v_aug_{q}")
        nc.vector.tensor_copy(va[:, 0:D], vq)
        nc.gpsimd.memset(va[:, D:D + 1], 1.0)
        v_aug.append(va)

    # ---- selector band [128, 132] bf16 -----------------------------------
    # band[c, y] = 1 iff y == 64 + (c >= 64).  The lhsT for local pair m is
    # band[:, 64-2m : 128-2m]: a [128, 64] matrix whose column p is 1 exactly
    # when p == 2m + t(c), t(c) = c // 64 -- so the matmul adds the d-sum of
    # half t of absd into output row 2m+t.  One static tensor, 32 shifted
    # views; matmul PSUM writes stay at legal base partitions {0, 64}.
    band = const.tile([P, 132], fp16, name="band")
    nc.gpsimd.memset(band[:], 0.0)
    nc.gpsimd.memset(band[0:D, D:D + 1], 1.0)
    nc.gpsimd.memset(band[D:2 * D, D + 1:D + 2], 1.0)


    # ---- main: distance + softmax + weighted sum ------------------------
    tr_ctx.close()
    psum_unn = ctx.enter_context(tc.tile_pool(name="psum_unn", bufs=4,
                                              space="PSUM"))
    psum_out = ctx.enter_context(tc.tile_pool(name="psum_out", bufs=1,
                                              space="PSUM"))
    out_aug = [psum_out.tile([P, D + 1], f32, name=f"out_aug_{qp}")
               for qp in range(NB)]

    # Banks 0..NB-2 on VectorE (tensor_scalar max); bank NB-1 on ScalarE
    # (Relu(v - k) = max(v,k) - k -- same V1-drop math, bias sign flipped).
    wts = [None] * NB
    unns = [None] * NB
    for q in range(NB):
        unn = psum_unn.tile([P, M], f32, name=f"unn_{q}", tag="unn")
        unns[q] = unn
        act_bank = (q == NB - 1)
        for h in range(2):  # half-bank: 64 j-rows at partition offset 64*h
            for m in range(32):
                mj = q * 64 + h * 32 + m
                if act_bank:
                    absd = act_pool.tile([P, M], fp16, name="absd_a",
                                         tag="absd_a")
                    nc.scalar.activation(absd[:], vT2[:], Act.Relu,
                                         bias=neg_k2T[:, mj:mj + 1],
                                         scale=1.0)
                else:
                    absd = absd_pool.tile([P, M], fp16, name="absd",
                                          tag="absd")
                    scl = 0.3 if cfg.get("ts_imm") else k2T[:, mj:mj + 1]
                    nc.vector.tensor_scalar(
                        absd[:], vT2[:], scl, None,
                        op0=Alu.max)
                nc.tensor.matmul(
                    unn[D * h:D * h + D, :], band[:, D - 2 * m:2 * D - 2 * m],
                    absd[:], start=(m == 0), stop=(m == 31))
    for q in range(NB):
        wT = wt_pool.tile([P, M], bf16, name="wT", tag="wT")
        wts[q] = wT
        nc.scalar.activation(wT[:], unns[q][:], Act.Exp, scale=1.0,
                             bias=bias_col[q][:])
    for q in range(NB):
        for qp in range(NB):
            nc.tensor.matmul(
                out_aug[qp][:], wts[q][:, qp * P:(qp + 1) * P], v_aug[q][:],
                start=(q == 0), stop=(q == NB - 1), skip_group_check=True)

    # ---- normalize + store ----------------------------------------------
    for qp in range(NB):
        recip = small.tile([P, 1], f32, name=f"recip_{qp}")
        nc.vector.reciprocal(recip[:], out_aug[qp][:, D:D + 1])
        res = small.tile([P, D], f32, name=f"res_{qp}")
        nc.vector.tensor_scalar(
            res[:], out_aug[qp][:, 0:D], recip[:], None, op0=Alu.mult)
        eng = [nc.sync, nc.scalar, nc.sync, nc.scalar][qp]
        eng.dma_start(out[qp * P:(qp + 1) * P, :], res[:])

    ctx.close()


def _get_module():
    if "nc" not in _CACHE:
        _CACHE["nc"] = _build_module()
    return _CACHE["nc"]


def _run(k, v, trace=False, tmpdir=None):
    """k, v: [B, M, D] f32. Returns (out [B, M, D] f32, BassKernelResults)."""
    from concourse import bass_utils

    nc = _get_module()
    kw = {"tmpdir": tmpdir} if tmpdir else {}
    in_maps = [
        {"k": np.ascontiguousarray(k[b], dtype=np.float32),
         "v": np.ascontiguousarray(v[b], dtype=np.float32)}
        for b in range(B)
    ]
    res = bass_utils.run_bass_kernel_spmd(
        nc, in_maps, core_ids=list(range(B)), trace=trace, **kw)
    out = np.stack([res.results[b]["out"] for b in range(B)], axis=0)
    return out, res


def kernel(**inputs):
    k = np.asarray(inputs["k"])
    v = np.asarray(inputs["v"])
    out, _ = _run(k, v, trace=bool(int(os.environ.get("KERNEL_TRACE", "0"))))
    return out.astype(np.float32)


# revision 19
# speedup vs baseline: 1.1407x; 1.0824x over previous
"""Laplace attention kernel for Trainium2 (8 NeuronCores, SPMD data-parallel).

Reference computation (per batch b):
    unnorm[i,j] = sum_d |(k[j,d] - v[i,d]) * 0.5|
    weights     = softmax_j(unnorm)          # rows i, softmax over j
    out[i,:]    = sum_j weights[i,j] * v[j,:]

B=8 batches -> one batch per NeuronCore, no cross-core communication.

Per-core algorithm (M=512, D=64, P=128):
  - Layouts:  vT2 [128=(t,d), 512=i] bf16 : v transposed, duplicated over t
              k2T [128=(t,d), 256=mj] f32 : column mj = [k[2mj,:]; k[2mj+1,:]]
  - For each j-pair mj: one DVE tensor_scalar
        absd[(t,d), i] = abs_max(vT2 - k2T[:,mj], 0) = |v[i,d] - k[2mj+t,d]|
    then one TensorE matmul with a constant selector lhsT [128,2]
    (column t selects the 64 d-rows of half t) reducing over d:
        unnT[2r+t, i] += ... -> PSUM bank q holds unnT rows j=128q..128q+127
    This produces unnorm TRANSPOSED ([j,i]), which is exactly the lhsT the
    final matmul needs -- no weight transpose anywhere.
  - Softmax without max-subtraction (values bounded, fp32 exp is safe):
        wT[j,i] = exp(0.5 * unnT[j,i])  (one ACT op per bank, PSUM->SBUF bf16)
  - Final matmul with v augmented by a ones column:
        out_aug[i, 0:64] = sum_j wT[j,i] * v[j,:],  out_aug[i,64] = sum_j wT[j,i]
    then out = out_aug[:, 0:64] * (1 / out_aug[:, 64]).
"""

import os

import numpy as np

M = 512
D = 64
B = 8
P = 128
NB = M // P  # 4 row-blocks
NMJ = M // 2  # 256 j-pairs
# Global shift on the softmax logits so exp() fits fp16 range (~[2e-14, 6e4]).
# Logits 0.5*sum_d|k-v| lie in ~[21, 55] for these inputs; weights are stored
# as exp(logit - 45); numerator and denominator scale identically so softmax
# ratios are unchanged.
EXP_SHIFT = 38.0

_CACHE = {}

# Experiment knobs (overridden by exp harness): dtype of the distance-phase
# 16-bit tensors and of the TS scalar operand.
CFG = {"mx_dt": "float16", "ts_imm": False}


def _build_module(cfg=None):
    import concourse.mybir as mybir
    import concourse.tile as tile
    from concourse import bacc

    nc = bacc.Bacc("TRN2", target_bir_lowering=False, debug=False,
                   enable_asserts=False)
    k_dram = nc.dram_tensor("k", [M, D], mybir.dt.float32, kind="ExternalInput")
    v_dram = nc.dram_tensor("v", [M, D], mybir.dt.float32, kind="ExternalInput")
    out_dram = nc.dram_tensor("out", [M, D], mybir.dt.float32,
                              kind="ExternalOutput")

    with tile.TileContext(nc) as tc:
        _emit(tc, nc, k_dram.ap(), v_dram.ap(), out_dram.ap(), cfg or CFG)
    nc.compile()
    return nc


def _emit(tc, nc, k, v, out, cfg):
    from contextlib import ExitStack

    import concourse.mybir as mybir
    from concourse.masks import make_identity

    f32 = mybir.dt.float32
    fp16 = getattr(mybir.dt, cfg.get("mx_dt", "float16"))
    bf16 = mybir.dt.bfloat16
    Alu = mybir.AluOpType
    Act = mybir.ActivationFunctionType

    ctx = ExitStack()
    const = ctx.enter_context(tc.tile_pool(name="const", bufs=1))
    absd_pool = ctx.enter_context(tc.tile_pool(name="absd", bufs=8))
    act_pool = ctx.enter_context(tc.tile_pool(name="absd_act", bufs=34))
    wt_pool = ctx.enter_context(tc.tile_pool(name="wt", bufs=4))
    small = ctx.enter_context(tc.tile_pool(name="small", bufs=1))
    # PSUM: tr pool (2 banks) is prep-only and released before the main-loop
    # pools (4 + 4 banks) are created, so peak stays at 8 banks.
    tr_ctx = ExitStack()
    psum_tr = tr_ctx.enter_context(tc.tile_pool(name="psum_tr", bufs=2,
                                                space="PSUM"))

    # ---- prep: identity for PE transposes -------------------------------
    ident = const.tile([P, P], f32, name="ident")
    make_identity(nc, ident)

    # ---- k2T [128, 256] f32: transpose of k.reshape(256, 128) -----------
    k2_view = k.rearrange("(h m t) d -> m h (t d)", t=2, h=2)  # [128, 2, 128]
    k2all = const.tile([P, 2, P], f32, name="k2all")
    nc.scalar.dma_start(k2all[:, 0, :], k2_view[:, 0, :])
    nc.sync.dma_start(k2all[:, 1, :], k2_view[:, 1, :])
    k2T = const.tile([P, NMJ], f32, name="k2T")
    ptrk = psum_tr.tile([P, 2 * P], f32, name="ptrk", tag="ptr")
    for h in range(2):
        nc.tensor.transpose(ptrk[:, h * P:(h + 1) * P], k2all[:, h, :],
                            ident[:])
    nc.scalar.copy(k2T[:], ptrk[:])
    neg_k2T = const.tile([P, NMJ], f32, name="neg_k2T")
    nc.vector.tensor_scalar(neg_k2T[:], k2T[:], -1.0, None, op0=Alu.mult)

    # ---- K1[j] = sum_d k[j,d], laid out [1, 512] as (t, mj) -------------
    # Needed because |a-b| = 2*max(a,b) - a - b:
    #   logit[j,i] = 0.5*sum_d|k-v| = Smax[j,i] - 0.5*K1[j] - 0.5*V1[i]
    # and the V1[i] term is constant along each softmax column -> cancels.
    half_sel = const.tile([P, 2], f32, name="half_sel")
    nc.gpsimd.memset(half_sel[:], 0.0)
    nc.gpsimd.memset(half_sel[0:D, 0:1], 1.0)
    nc.gpsimd.memset(half_sel[D:2 * D, 1:2], 1.0)
    k1_psum = psum_tr.tile([1, M], f32, name="k1_psum", tag="ptr")
    for t in range(2):
        nc.tensor.matmul(k1_psum[0:1, t * NMJ:(t + 1) * NMJ],
                         half_sel[:, t:t + 1], k2T[:],
                         start=True, stop=True)
    # Copy PSUM (t-major) -> SBUF j-major [1, 512]: out position 2*mj + t.
    k1_rows = const.tile([1, M], f32, name="k1_rows")
    nc.scalar.copy(k1_rows.rearrange("p (mj t) -> p t mj", t=2)[:],
                   k1_psum.rearrange("p (t mj) -> p t mj", t=2)[:])
    # Per-bank j-major [128, 1] bias columns via PE transpose.
    bias_col = []
    for q in range(NB):
        trc = psum_tr.tile([P, 1], f32, name=f"trc_{q}", tag="ptr")
        nc.tensor.transpose(trc[:], k1_rows[0:1, q * P:(q + 1) * P],
                            ident[0:1, 0:1])
        bc = const.tile([P, 1], f32, name=f"bias_{q}")
        if q == NB - 1:
            nc.vector.tensor_scalar(bc[0:D, :], trc[0:D, :], -0.5, -EXP_SHIFT,
                                    op0=Alu.mult, op1=Alu.add)
            nc.vector.tensor_scalar(bc[D:2 * D, :], trc[D:2 * D, :], 0.5,
                                    -EXP_SHIFT, op0=Alu.mult, op1=Alu.add)
        else:
            nc.vector.tensor_scalar(bc[:], trc[:], -0.5, -EXP_SHIFT,
                                    op0=Alu.mult, op1=Alu.add)
        bias_col.append(bc)

    # ---- v tiles, vT2 [128, 512] bf16, v_aug [128, 65] bf16 x4 ----------
    vT2 = const.tile([P, M], fp16, name="vT2")
    v4 = const.tile([P, NB, D], f32, name="v4")
    v_view = v.rearrange("(q p) d -> p q d", p=P)
    nc.scalar.dma_start(v4[:, 0:2, :], v_view[:, 0:2, :])
    nc.sync.dma_start(v4[:, 2:4, :], v_view[:, 2:4, :])
    ptv = psum_tr.tile([D, M], f32, name="ptv", tag="ptr")
    v_aug = []
    for q in range(NB):
        nc.tensor.transpose(ptv[:, q * P:(q + 1) * P], v4[:, q, :], ident[:])
        va = const.tile([P, D + 1], bf16, name=f"v_aug_{q}")
        nc.scalar.copy(va[:, 0:D], v4[:, q, :])
        nc.gpsimd.memset(va[:, D:D + 1], 1.0)
        v_aug.append(va)
    # copy vT into both t-halves of vT2 (cast f32 -> fp16) on DVE
    nc.vector.tensor_copy(vT2[0:D, :], ptv[:])
    nc.vector.tensor_copy(vT2[D:2 * D, :], ptv[:])

    # ---- selector band [128, 132] bf16 -----------------------------------
    # band[c, y] = 1 iff y == 64 + (c >= 64).  The lhsT for local pair m is
    # band[:, 64-2m : 128-2m]: a [128, 64] matrix whose column p is 1 exactly
    # when p == 2m + t(c), t(c) = c // 64 -- so the matmul adds the d-sum of
    # half t of absd into output row 2m+t.  One static tensor, 32 shifted
    # views; matmul PSUM writes stay at legal base partitions {0, 64}.
    band = const.tile([P, 132], fp16, name="band")
    nc.gpsimd.memset(band[:], 0.0)
    nc.gpsimd.memset(band[0:D, D:D + 1], 1.0)
    nc.gpsimd.memset(band[D:2 * D, D + 1:D + 2], 1.0)


    # ---- main: distance + softmax + weighted sum ------------------------
    tr_ctx.close()
    psum_unn = ctx.enter_context(tc.tile_pool(name="psum_unn", bufs=4,
                                              space="PSUM"))
    psum_out = ctx.enter_context(tc.tile_pool(name="psum_out", bufs=1,
                                              space="PSUM"))
    out_aug = [psum_out.tile([P, D + 1], f32, name=f"out_aug_{qp}")
               for qp in range(NB)]

    # Banks 0..NB-2 on VectorE (tensor_scalar max); bank NB-1 on ScalarE
    # (Relu(v - k) = max(v,k) - k -- same V1-drop math, bias sign flipped).
    wts = [None] * NB
    unns = [None] * NB
    for q in range(NB):
        unn = psum_unn.tile([P, M], f32, name=f"unn_{q}", tag="unn")
        unns[q] = unn
        # Interleave the two half-bank accumulation chains so consecutive
        # matmuls target independent PSUM regions (no accumulate RAW stall).
        for step in range(64):
            h, m = step % 2, step // 2
            mj = q * 64 + h * 32 + m
            on_act = (q == NB - 1 and h == 1)
            if on_act:
                absd = act_pool.tile([P, M], fp16, name="absd_a",
                                     tag="absd_a")
                nc.scalar.activation(absd[:], vT2[:], Act.Relu,
                                     bias=neg_k2T[:, mj:mj + 1],
                                     scale=1.0)
            else:
                absd = absd_pool.tile([P, M], fp16, name="absd",
                                      tag="absd")
                scl = 0.3 if cfg.get("ts_imm") else k2T[:, mj:mj + 1]
                nc.vector.tensor_scalar(
                    absd[:], vT2[:], scl, None,
                    op0=Alu.max)
            nc.tensor.matmul(
                unn[D * h:D * h + D, :], band[:, D - 2 * m:2 * D - 2 * m],
                absd[:], start=(m == 0), stop=(m == 31),
                skip_group_check=True)
    for q in range(NB):
        wT = wt_pool.tile([P, M], bf16, name="wT", tag="wT")
        wts[q] = wT
        nc.scalar.activation(wT[:], unns[q][:], Act.Exp, scale=1.0,
                             bias=bias_col[q][:])
    for q in range(NB):
        for qp in range(NB):
            nc.tensor.matmul(
                out_aug[qp][:], wts[q][:, qp * P:(qp + 1) * P], v_aug[q][:],
                start=(q == 0), stop=(q == NB - 1), skip_group_check=True)

    # ---- normalize + store ----------------------------------------------
    for qp in range(NB):
        recip = small.tile([P, 1], f32, name=f"recip_{qp}")
        nc.vector.reciprocal(recip[:], out_aug[qp][:, D:D + 1])
        res = small.tile([P, D], f32, name=f"res_{qp}")
        nc.vector.tensor_scalar(
            res[:], out_aug[qp][:, 0:D], recip[:], None, op0=Alu.mult)
        eng = [nc.sync, nc.scalar, nc.sync, nc.scalar][qp]
        eng.dma_start(out[qp * P:(qp + 1) * P, :], res[:])

    ctx.close()


def _get_module():
    if "nc" not in _CACHE:
        _CACHE["nc"] = _build_module()
    return _CACHE["nc"]


def _run(k, v, trace=False, tmpdir=None):
    """k, v: [B, M, D] f32. Returns (out [B, M, D] f32, BassKernelResults)."""
    from concourse import bass_utils

    nc = _get_module()
    kw = {"tmpdir": tmpdir} if tmpdir else {}
    in_maps = [
        {"k": np.ascontiguousarray(k[b], dtype=np.float32),
         "v": np.ascontiguousarray(v[b], dtype=np.float32)}
        for b in range(B)
    ]
    res = bass_utils.run_bass_kernel_spmd(
        nc, in_maps, core_ids=list(range(B)), trace=trace, **kw)
    out = np.stack([res.results[b]["out"] for b in range(B)], axis=0)
    return out, res


def kernel(**inputs):
    k = np.asarray(inputs["k"])
    v = np.asarray(inputs["v"])
    out, _ = _run(k, v, trace=bool(int(os.environ.get("KERNEL_TRACE", "0"))))
    return out.astype(np.float32)
